# revision 1
# baseline (speedup 1.0000x reference)
"""Trainium2 Bass kernel: batched inverse of homogeneous affine transforms.

Problem: trf (B, 3, 4) fp32 "shift" affines. Padded M = [[I3 + dA, t], [0, 1]].
Output = top 3 rows of M^-1 = [A^-1 | -A^-1 t] where A = I3 + dA.

Closed form via the column-cross-product adjugate:
    inv(A) row r = (1/det) * cross(a_{r+1}, a_{r+2})   (columns a1,a2,a3, cyclic)
    det          = a1 . cross(a2, a3)
    col3_r       = -sum_j inv(A)[r, j] * t_j

Everything is elementwise over the batch -> memory-bound. The batch is
sharded over 8 NeuronCores; each core streams (BL, 12) fp32 in and out.

Per-core layout: chunks of 128 partitions x C matrices; the SBUF input tile
is (128, 12*C) with each partition holding C consecutive 12-float matrices.
All compute uses strided/broadcast access patterns directly on the
interleaved layout (fp32 tensor ops on DVE run at 1x regardless of stride).
Work is split across DVE (products/scale), GPSIMD (contiguous adds/subs)
and ACT (diag +1, reciprocal).
"""

import numpy as np

B = 4_194_304
NCORES = 8
BL = B // NCORES  # 524288 matrices per core
P = 128
C = 512  # matrices per partition per chunk


def _V(base_ap, off, dims):
    """Build a strided view of a tile: dims = [(step, count), ...] free dims,
    iterated with the LAST dim innermost. Offset in elements."""
    import concourse.bass as bass

    return bass.AP(
        base_ap.tensor,
        base_ap.offset + off,
        [list(base_ap.ap[0])] + [[int(s), int(n)] for s, n in dims],
    )


# default engine plan: op -> "v" (DVE) / "g" (GPSIMD)
DEFAULT_PLAN = {
    **{f"prod{k}": "v" for k in range(18)},
    "zsub": "g",
    "tm": "g",
    "det1": "g",
    "det2": "g",
    "scale9": "v",
    "w": "v",
    "s1": "g",
    "s2": "g",
}

# Products: (left position, right position) in the 12-float group.
# Positions: a=0 b=1 c=2 t0=3 d=4 e=5 f=6 t1=7 g=8 h=9 i=10 t2=11
# P[3r+j] = x_r[(j+1)%3] * y_r[(j+2)%3], Q[3r+j] = x_r[(j+2)%3] * y_r[(j+1)%3]
# with (x_r, y_r) = (a2,a3), (a3,a1), (a1,a2); cols a1=(0,4,8) a2=(1,5,9) a3=(2,6,10)
PRODS = [
    (5, 10), (9, 2), (1, 6),    # P, r=0: cross(a2,a3)
    (6, 8), (10, 0), (2, 4),    # P, r=1: cross(a3,a1)
    (4, 9), (8, 1), (0, 5),     # P, r=2: cross(a1,a2)
    (9, 6), (1, 10), (5, 2),    # Q, r=0
    (10, 4), (2, 8), (6, 0),    # Q, r=1
    (8, 5), (0, 9), (4, 1),     # Q, r=2
]


def build_nc(bl=BL, c=C, plan=None):
    import concourse.bass as bass
    import concourse.bacc as bacc
    import concourse.mybir as mybir
    from concourse.tile import TileContext

    plan = dict(DEFAULT_PLAN, **(plan or {}))
    f32 = mybir.dt.float32
    nch = bl // (P * c)
    assert bl == nch * P * c

    # Bacc (not plain Bass): Tile emits multi-wait instructions; Bacc's
    # generate_event_semaphores splits them to satisfy TRN2's 1-wait limit.
    nc = bacc.Bacc()
    trf = nc.declare_dram_parameter("trf", [bl, 12], f32, isOutput=False)
    out = nc.declare_dram_parameter("out", [bl, 12], f32, isOutput=True)
    trf_t = trf.ap().rearrange("(n p c) m -> n p (c m)", p=P, c=c)
    out_t = out.ap().rearrange("(n p c) m -> n p (c m)", p=P, c=c)

    with TileContext(nc) as tc:
        with (
            tc.tile_pool(name="io", bufs=2) as io,
            tc.tile_pool(name="tmp", bufs=1) as tmp,
        ):
            for n in range(nch):
                eng = {"v": nc.vector, "g": nc.gpsimd}

                tin = io.tile([P, 12 * c], f32, tag="tin")
                nc.sync.dma_start(out=tin[:], in_=trf_t[n])

                # diag += 1 in-place: positions {0,5,10} = stride 5
                dg = _V(tin, 0, [(12, c), (5, 3)])
                nc.scalar.add(dg, dg, 1.0)

                # P/Q products: pq planes 0-8 = P (cross components Z before
                # subtraction), planes 9-17 = Q; plane k = C contiguous floats
                pq = tmp.tile([P, 18 * c], f32, tag="pq")
                for k, (l, r) in enumerate(PRODS):
                    e = eng[plan[f"prod{k}"]]
                    e.tensor_mul(
                        _V(pq, k * c, [(1, c)]),
                        _V(tin, l, [(12, c)]),
                        _V(tin, r, [(12, c)]),
                    )

                # Z = P - Q (in place over P), flat 9C, contiguous
                pf = _V(pq, 0, [(1, 9 * c)])
                qf = _V(pq, 9 * c, [(1, 9 * c)])
                eng[plan["zsub"]].tensor_sub(pf, pf, qf)

                # det = a1 . Z[0:3]:  tm = a1 * Z3 ; det = tm0+tm1+tm2
                tm = tmp.tile([P, 3 * c], f32, tag="tm")
                # iteration (k, c): in0 strided tin cols, in1 Z planes, out tm
                eng[plan["tm"]].tensor_mul(
                    _V(tm, 0, [(c, 3), (1, c)]),
                    _V(tin, 0, [(4, 3), (12, c)]),
                    _V(pq, 0, [(c, 3), (1, c)]),
                )
                det = tmp.tile([P, c], f32, tag="det")
                eng[plan["det1"]].tensor_add(
                    det[:], _V(tm, 0, [(1, c)]), _V(tm, c, [(1, c)])
                )
                eng[plan["det2"]].tensor_add(det[:], det[:], _V(tm, 2 * c, [(1, c)]))

                # rdet = 1/det: ~2 ULP, two custom-DVE ops (det ~ 1, no edge
                # cases). Replicated to 3 planes (ISA ops are <=3D and don't
                # take 0-step broadcast APs).
                rdet3 = tmp.tile([P, 3 * c], f32, tag="rdet3")
                rscr = tmp.tile([P, c], f32, tag="rscr")
                nc.vector.reciprocal_approx_accurate(
                    _V(rdet3, 0, [(1, c)]), det[:], rscr[:]
                )
                nc.scalar.copy(_V(rdet3, c, [(1, c)]), _V(rdet3, 0, [(1, c)]))
                nc.scalar.copy(_V(rdet3, 2 * c, [(1, c)]), _V(rdet3, 0, [(1, c)]))

                # out 3x3 block: tout[4r+j] = Z[3r+j] * rdet  (one op per row,
                # iteration (c, j), all operands 3D)
                tout = io.tile([P, 12 * c], f32, tag="tout")
                for r in range(3):
                    eng[plan["scale9"]].tensor_mul(
                        _V(tout, 4 * r, [(12, c), (1, 3)]),
                        _V(pq, 3 * r * c, [(1, c), (c, 3)]),
                        _V(rdet3, 0, [(1, c), (c, 3)]),
                    )

                # W[r,j] = (tout[4r+j] * -1) * t_j  (scalar_tensor_tensor,
                # one per row); W lives in the dead Q region
                for r in range(3):
                    eng[plan["w"]].scalar_tensor_tensor(
                        _V(pq, (9 + 3 * r) * c, [(1, c), (c, 3)]),
                        _V(tout, 4 * r, [(12, c), (1, 3)]),
                        -1.0,
                        _V(tin, 3, [(12, c), (4, 3)]),
                        mybir.AluOpType.mult,
                        mybir.AluOpType.mult,
                    )

                # col3_r = W[r,0] + W[r,1] + W[r,2] -> tout positions {3,7,11}
                s = tmp.tile([P, 3 * c], f32, tag="s")
                eng[plan["s1"]].tensor_add(
                    _V(s, 0, [(c, 3), (1, c)]),
                    _V(pq, 9 * c, [(3 * c, 3), (1, c)]),
                    _V(pq, 10 * c, [(3 * c, 3), (1, c)]),
                )
                eng[plan["s2"]].tensor_add(
                    _V(tout, 3, [(4, 3), (12, c)]),
                    _V(s, 0, [(c, 3), (1, c)]),
                    _V(pq, 11 * c, [(3 * c, 3), (1, c)]),
                )

                nc.sync.dma_start(out=out_t[n], in_=tout[:])

    return nc


_CACHE = {}


def _get_nc():
    if "nc" not in _CACHE:
        nc = build_nc()
        # Bacc.finalize runs the bacc pipeline (event-sem wait splitting,
        # register allocation, ...); the PJRT path executes it as-is.
        nc.finalize()
        _CACHE["nc"] = nc
    return _CACHE["nc"]


def run(trf, trace=False, **spmd_kwargs):
    """Shard, run on 8 cores, gather. Returns (output, BassKernelResults)."""
    from concourse.bass_utils import run_bass_kernel_spmd

    x = np.ascontiguousarray(np.asarray(trf, dtype=np.float32)).reshape(NCORES, BL, 12)
    in_maps = [{"trf": x[i]} for i in range(NCORES)]
    nc = _get_nc()
    res = run_bass_kernel_spmd(
        nc, in_maps, list(range(NCORES)), trace=trace, **spmd_kwargs
    )
    outs = np.stack([np.asarray(res.results[i]["out"]) for i in range(NCORES)])
    return outs.reshape(B, 3, 4).astype(np.float32), res


def kernel(trf):
    return run(trf)[0]



# revision 2
# speedup vs baseline: 1.3076x; 1.3076x over previous
"""Trainium2 Bass kernel: batched inverse of homogeneous affine transforms.

Problem: trf (B, 3, 4) fp32 "shift" affines. Padded M = [[I3 + dA, t], [0, 1]].
Output = top 3 rows of M^-1 = [A^-1 | -A^-1 t] where A = I3 + dA.

Closed form via the column-cross-product adjugate:
    Z[3r+j]  = P[3r+j] - Q[3r+j]   (cross(a_{r+1}, a_{r+2}) components)
    det      = a1 . Z[0:3] ; O = Z * (1/det) ; w_r = sum_j O[r][j] * (-t_j)

Layout: PLANAR per partition. Host pre-permutes each core's (BL, 12) slab to
(nch, 128, 12, C): partition p holds 12 contiguous planes of C consecutive
matrices. Every engine op then runs on dense step-1 inner runs (measured ~2x
faster on DVE than the stride-12 interleaved layout) while each DMA still
moves one contiguous 24KB run per partition.

Plane permutation POS (slot -> matrix position) was chosen by combinatorial
search so the 9 Q-products batch as 3 ops, P row 0 batches, and tm batches
(arithmetic-progression plane strides). Work is split DVE / GPSIMD / ACT to
balance engine busy time; all pools are double-buffered so chunks pipeline.
"""

import numpy as np

B = 4_194_304
NCORES = 8
BL = B // NCORES  # 524288 matrices per core
P = 128
C = 512           # matrices per partition per chunk
NCH = BL // (P * C)  # 8 chunks

# slot -> input position (position = 4*r + col, row-major (3,4))
POS = [5, 0, 4, 8, 9, 10, 2, 6, 1, 3, 7, 11]
# output plane k -> output position: planes 0..8 = O[r][j] at 4r+j, 9..11 = w_r
OPOS = [0, 1, 2, 4, 5, 6, 8, 9, 10, 3, 7, 11]

# P products (out plane 3r+j in po block), as (out, in0_slot, in1_slot):
# row 0 batched: out {0,1,2}, in0 [0,4,8] (step 4), in1 [5,6,7] (step 1)
P_SINGLES = [
    (3, 7, 3), (4, 5, 1), (5, 6, 2),   # row 1
    (6, 2, 4), (7, 3, 8), (8, 1, 0),   # row 2
]
# Q products batched by j: (out_base, out_step, in0_base, in0_step, in1_base, in1_step)
Q_BATCH = [
    (0, 3, 7, -2, 4, -2),   # j=0: out {0,3,6}, in0 [7,5,3], in1 [4,2,0]
    (1, 3, 5, -2, 8, -2),   # j=1: out {1,4,7}, in0 [5,3,1], in1 [8,6,4]
    (2, 3, 0, 1, 6, 1),     # j=2: out {2,5,8}, in0 [0,1,2], in1 [6,7,8]
]

# engine plan: op -> "v" (DVE) / "g" (GPSIMD)
DEFAULT_PLAN = {
    "p0": "v",                               # batched P row 0
    **{f"ps{i}": "v" for i in range(6)},     # P singles
    **{f"q{j}": "v" for j in range(3)},      # batched Q
    "z": "v",
    "tm": "v",
    "det1": "v",
    "det2": "v",
    "s": "v",
    "w": "v",
    **{f"scale{r}": "g" for r in range(3)},
    **{f"wp{r}": "g" for r in range(3)},
}


def _V(base_ap, off, dims):
    """Strided view of a tile: dims = [(step, count), ...] free dims, last
    dim innermost. Offset in elements."""
    import concourse.bass as bass

    return bass.AP(
        base_ap.tensor,
        base_ap.offset + off,
        [list(base_ap.ap[0])] + [[int(s), int(n)] for s, n in dims],
    )


def build_nc(bl=BL, c=C, plan=None):
    import concourse.bass as bass
    import concourse.bacc as bacc
    import concourse.mybir as mybir
    from concourse.tile import TileContext

    plan = dict(DEFAULT_PLAN, **(plan or {}))
    f32 = mybir.dt.float32
    nch = bl // (P * c)
    assert bl == nch * P * c

    nc = bacc.Bacc()
    # DRAM layout (host-permuted): (nch*128, 12*C) — row = (chunk, partition),
    # 12 planar planes of C floats contiguous per row.
    trf = nc.declare_dram_parameter("trf", [nch * P, 12 * c], f32, isOutput=False)
    out = nc.declare_dram_parameter("out", [nch * P, 12 * c], f32, isOutput=True)
    trf_t = trf.ap().rearrange("(n p) f -> n p f", p=P)
    out_t = out.ap().rearrange("(n p) f -> n p f", p=P)

    with TileContext(nc) as tc:
        with (
            tc.tile_pool(name="io", bufs=2) as io,
            tc.tile_pool(name="tmp", bufs=2) as tmp,
        ):
            for n in range(nch):
                eng = {"v": nc.vector, "g": nc.gpsimd}

                t = io.tile([P, 12 * c], f32, tag="t")
                nc.sync.dma_start(out=t[:], in_=trf_t[n])

                # diag += 1: slots {0,1} and {5}
                d01 = _V(t, 0, [(1, 2 * c)])
                nc.scalar.add(d01, d01, 1.0)
                d5 = _V(t, 5 * c, [(1, c)])
                nc.scalar.add(d5, d5, 1.0)

                po = io.tile([P, 12 * c], f32, tag="po")  # P/Z/O planes 0..8, w 9..11
                qq = tmp.tile([P, 9 * c], f32, tag="qq")  # Q then wp

                # P row 0 batched
                eng[plan["p0"]].tensor_mul(
                    _V(po, 0, [(c, 3), (1, c)]),
                    _V(t, 0, [(4 * c, 3), (1, c)]),
                    _V(t, 5 * c, [(c, 3), (1, c)]),
                )
                # P singles (rows 1, 2)
                for i, (k, a, b) in enumerate(P_SINGLES):
                    eng[plan[f"ps{i}"]].tensor_mul(
                        _V(po, k * c, [(1, c)]),
                        _V(t, a * c, [(1, c)]),
                        _V(t, b * c, [(1, c)]),
                    )
                # Q batched by j
                for j, (ob, os_, a0, s0, b0, s1) in enumerate(Q_BATCH):
                    eng[plan[f"q{j}"]].tensor_mul(
                        _V(qq, ob * c, [(os_ * c, 3), (1, c)]),
                        _V(t, a0 * c, [(s0 * c, 3), (1, c)]),
                        _V(t, b0 * c, [(s1 * c, 3), (1, c)]),
                    )

                # Z = P - Q (in place over P block), flat 9C
                pf = _V(po, 0, [(1, 9 * c)])
                eng[plan["z"]].tensor_sub(pf, pf, _V(qq, 0, [(1, 9 * c)]))

                # tm[j] = a1_j * Z_j: in0 slots [1,2,3], in1 Z planes [0,1,2]
                tm = tmp.tile([P, 3 * c], f32, tag="tm")
                eng[plan["tm"]].tensor_mul(
                    _V(tm, 0, [(c, 3), (1, c)]),
                    _V(t, c, [(c, 3), (1, c)]),
                    _V(po, 0, [(c, 3), (1, c)]),
                )
                det = tmp.tile([P, c], f32, tag="det")
                eng[plan["det1"]].tensor_add(
                    det[:], _V(tm, 0, [(1, c)]), _V(tm, c, [(1, c)])
                )
                eng[plan["det2"]].tensor_add(det[:], det[:], _V(tm, 2 * c, [(1, c)]))

                # rdet = 1/det (2 custom DVE ops), replicated to 3 planes on ACT
                rdet3 = tmp.tile([P, 3 * c], f32, tag="rdet3")
                rscr = tmp.tile([P, c], f32, tag="rscr")
                nc.vector.reciprocal_approx_accurate(
                    _V(rdet3, 0, [(1, c)]), det[:], rscr[:]
                )
                nc.scalar.copy(_V(rdet3, c, [(1, c)]), _V(rdet3, 0, [(1, c)]))
                nc.scalar.copy(_V(rdet3, 2 * c, [(1, c)]), _V(rdet3, 0, [(1, c)]))

                # tneg: t planes 9..11 *= -1 (in place, ACT)
                tp = _V(t, 9 * c, [(1, 3 * c)])
                nc.scalar.mul(tp, tp, -1.0)

                # O row r = Z row r * rdet (in place over Z), flat 3C each
                for r in range(3):
                    eng[plan[f"scale{r}"]].tensor_mul(
                        _V(po, 3 * r * c, [(1, 3 * c)]),
                        _V(po, 3 * r * c, [(1, 3 * c)]),
                        _V(rdet3, 0, [(1, 3 * c)]),
                    )
                # wp row r = O row r * (-t), overwrites Q block
                for r in range(3):
                    eng[plan[f"wp{r}"]].tensor_mul(
                        _V(qq, 3 * r * c, [(1, 3 * c)]),
                        _V(po, 3 * r * c, [(1, 3 * c)]),
                        _V(t, 9 * c, [(1, 3 * c)]),
                    )

                # w_r = wp[3r] + wp[3r+1] + wp[3r+2] -> po planes 9..11
                s = tmp.tile([P, 3 * c], f32, tag="s")
                eng[plan["s"]].tensor_add(
                    _V(s, 0, [(c, 3), (1, c)]),
                    _V(qq, 0, [(3 * c, 3), (1, c)]),
                    _V(qq, c, [(3 * c, 3), (1, c)]),
                )
                eng[plan["w"]].tensor_add(
                    _V(po, 9 * c, [(c, 3), (1, c)]),
                    _V(s, 0, [(c, 3), (1, c)]),
                    _V(qq, 2 * c, [(3 * c, 3), (1, c)]),
                )

                nc.sync.dma_start(out=out_t[n], in_=po[:])

    return nc


_CACHE = {}


def _get_nc():
    if "nc" not in _CACHE:
        nc = build_nc()
        nc.finalize()
        _CACHE["nc"] = nc
    return _CACHE["nc"]


def _shard_inputs(trf):
    """(B,3,4) -> per-core (nch*128, 12*C) planar slabs."""
    x = np.ascontiguousarray(np.asarray(trf, dtype=np.float32)).reshape(
        NCORES, NCH, P, C, 12
    )
    # permute matrix positions into plane slots, planes outer, matrices inner
    xp = x[:, :, :, :, POS].transpose(0, 1, 2, 4, 3)  # (8, nch, 128, 12, C)
    xp = np.ascontiguousarray(xp).reshape(NCORES, NCH * P, 12 * C)
    return xp


def _unshard_output(outs):
    """per-core (nch*128, 12*C) planar -> (B, 3, 4)."""
    o = outs.reshape(NCORES, NCH, P, 12, C).transpose(0, 1, 2, 4, 3)
    full = np.empty((NCORES, NCH, P, C, 12), dtype=np.float32)
    full[..., OPOS] = o
    return full.reshape(B, 3, 4)


def run(trf, trace=False, **spmd_kwargs):
    """Shard, run on 8 cores, gather. Returns (output, BassKernelResults)."""
    from concourse.bass_utils import run_bass_kernel_spmd

    xp = _shard_inputs(trf)
    in_maps = [{"trf": xp[i]} for i in range(NCORES)]
    nc = _get_nc()
    res = run_bass_kernel_spmd(
        nc, in_maps, list(range(NCORES)), trace=trace, **spmd_kwargs
    )
    outs = np.stack([np.asarray(res.results[i]["out"]) for i in range(NCORES)])
    return _unshard_output(outs).astype(np.float32), res


def kernel(trf):
    return run(trf)[0]


# revision 3
# speedup vs baseline: 1.6714x; 1.2782x over previous
"""Trainium2 Bass kernel: batched inverse of homogeneous affine transforms.

Problem: trf (B, 3, 4) fp32 "shift" affines. Padded M = [[I3 + dA, t], [0, 1]].
Output = top 3 rows of M^-1 = [A^-1 | -A^-1 t] where A = I3 + dA.

Closed form via the column-cross-product adjugate:
    Z[3r+j]  = P[3r+j] - Q[3r+j]   (cross(a_{r+1}, a_{r+2}) components)
    det      = a1 . Z[0:3] ; O = Z * (1/det) ; w_r = sum_j O[r][j] * (-t_j)

Layout: PLANAR per partition. Host pre-permutes each core's (BL, 12) slab to
(nch, 128, 12, C): partition p holds 12 contiguous planes of C consecutive
matrices. Every engine op then runs on dense step-1 inner runs (measured ~2x
faster on DVE than the stride-12 interleaved layout) while each DMA still
moves one contiguous 24KB run per partition.

Plane permutation POS (slot -> matrix position) was chosen by combinatorial
search so the 9 Q-products batch as 3 ops, P row 0 batches, and tm batches
(arithmetic-progression plane strides). Work is split DVE / GPSIMD / ACT to
balance engine busy time; all pools are double-buffered so chunks pipeline.
"""

import numpy as np

B = 4_194_304
NCORES = 8
BL = B // NCORES  # 524288 matrices per core
P = 128
C = 512           # matrices per partition per chunk
NCH = BL // (P * C)  # 8 chunks

# slot -> input position (position = 4*r + col, row-major (3,4))
POS = [5, 0, 4, 8, 9, 10, 2, 6, 1, 3, 7, 11]
# output plane k -> output position: planes 0..8 = O[r][j] at 4r+j, 9..11 = w_r
OPOS = [0, 1, 2, 4, 5, 6, 8, 9, 10, 3, 7, 11]

# P products (out plane 3r+j in po block), as (out, in0_slot, in1_slot):
# row 0 batched: out {0,1,2}, in0 [0,4,8] (step 4), in1 [5,6,7] (step 1)
P_SINGLES = [
    (3, 7, 3), (4, 5, 1), (5, 6, 2),   # row 1
    (6, 2, 4), (7, 3, 8), (8, 1, 0),   # row 2
]
# Q products batched by j: (out_base, out_step, in0_base, in0_step, in1_base, in1_step)
Q_BATCH = [
    (0, 3, 7, -2, 4, -2),   # j=0: out {0,3,6}, in0 [7,5,3], in1 [4,2,0]
    (1, 3, 5, -2, 8, -2),   # j=1: out {1,4,7}, in0 [5,3,1], in1 [8,6,4]
    (2, 3, 0, 1, 6, 1),     # j=2: out {2,5,8}, in0 [0,1,2], in1 [6,7,8]
]

# engine plan: op -> "v" (DVE) / "g" (GPSIMD)
DEFAULT_PLAN = {
    "p0": "v",                               # batched P row 0
    **{f"ps{i}": "v" for i in range(6)},     # P singles
    **{f"q{j}": "v" for j in range(3)},      # batched Q
    "z": "v",
    "tm": "v",
    "det1": "v",
    "det2": "v",
    "s": "v",
    "w": "v",
    **{f"scale{r}": "v" for r in range(3)},
    **{f"wp{r}": "v" for r in range(3)},
}


def _V(base_ap, off, dims):
    """Strided view of a tile: dims = [(step, count), ...] free dims, last
    dim innermost. Offset in elements."""
    import concourse.bass as bass

    return bass.AP(
        base_ap.tensor,
        base_ap.offset + off,
        [list(base_ap.ap[0])] + [[int(s), int(n)] for s, n in dims],
    )


def build_nc(bl=BL, c=C, plan=None):
    import concourse.bass as bass
    import concourse.bacc as bacc
    import concourse.mybir as mybir
    from concourse.tile import TileContext

    plan = dict(DEFAULT_PLAN, **(plan or {}))
    f32 = mybir.dt.float32
    nch = bl // (P * c)
    assert bl == nch * P * c

    nc = bacc.Bacc()
    # DRAM layout (host-permuted): (nch*128, 12*C) — row = (chunk, partition),
    # 12 planar planes of C floats contiguous per row.
    trf = nc.declare_dram_parameter("trf", [nch * P, 12 * c], f32, isOutput=False)
    out = nc.declare_dram_parameter("out", [nch * P, 12 * c], f32, isOutput=True)
    trf_t = trf.ap().rearrange("(n p) f -> n p f", p=P)
    out_t = out.ap().rearrange("(n p) f -> n p f", p=P)

    with TileContext(nc) as tc:
        with (
            tc.tile_pool(name="io", bufs=2) as io,
            tc.tile_pool(name="tmp", bufs=2) as tmp,
        ):
            for n in range(nch):
                eng = {"v": nc.vector, "g": nc.gpsimd}

                t = io.tile([P, 12 * c], f32, tag="t")
                nc.sync.dma_start(out=t[:], in_=trf_t[n])

                # diag += 1: slots {0,1} and {5}
                d01 = _V(t, 0, [(1, 2 * c)])
                nc.scalar.add(d01, d01, 1.0)
                d5 = _V(t, 5 * c, [(1, c)])
                nc.scalar.add(d5, d5, 1.0)

                po = io.tile([P, 12 * c], f32, tag="po")  # P/Z/O planes 0..8, w 9..11
                qq = tmp.tile([P, 9 * c], f32, tag="qq")  # Q then wp

                # P row 0 batched
                eng[plan["p0"]].tensor_mul(
                    _V(po, 0, [(c, 3), (1, c)]),
                    _V(t, 0, [(4 * c, 3), (1, c)]),
                    _V(t, 5 * c, [(c, 3), (1, c)]),
                )
                # P singles (rows 1, 2)
                for i, (k, a, b) in enumerate(P_SINGLES):
                    eng[plan[f"ps{i}"]].tensor_mul(
                        _V(po, k * c, [(1, c)]),
                        _V(t, a * c, [(1, c)]),
                        _V(t, b * c, [(1, c)]),
                    )
                # Q batched by j
                for j, (ob, os_, a0, s0, b0, s1) in enumerate(Q_BATCH):
                    eng[plan[f"q{j}"]].tensor_mul(
                        _V(qq, ob * c, [(os_ * c, 3), (1, c)]),
                        _V(t, a0 * c, [(s0 * c, 3), (1, c)]),
                        _V(t, b0 * c, [(s1 * c, 3), (1, c)]),
                    )

                # Z = P - Q (in place over P block), flat 9C
                pf = _V(po, 0, [(1, 9 * c)])
                eng[plan["z"]].tensor_sub(pf, pf, _V(qq, 0, [(1, 9 * c)]))

                # tm[j] = a1_j * Z_j: in0 slots [1,2,3], in1 Z planes [0,1,2]
                tm = tmp.tile([P, 3 * c], f32, tag="tm")
                eng[plan["tm"]].tensor_mul(
                    _V(tm, 0, [(c, 3), (1, c)]),
                    _V(t, c, [(c, 3), (1, c)]),
                    _V(po, 0, [(c, 3), (1, c)]),
                )
                det = tmp.tile([P, c], f32, tag="det")
                eng[plan["det1"]].tensor_add(
                    det[:], _V(tm, 0, [(1, c)]), _V(tm, c, [(1, c)])
                )
                eng[plan["det2"]].tensor_add(det[:], det[:], _V(tm, 2 * c, [(1, c)]))

                # rdet = 1/det (2 custom DVE ops), replicated to 3 planes on ACT
                rdet3 = tmp.tile([P, 3 * c], f32, tag="rdet3")
                rscr = tmp.tile([P, c], f32, tag="rscr")
                nc.vector.reciprocal_approx_accurate(
                    _V(rdet3, 0, [(1, c)]), det[:], rscr[:]
                )
                nc.scalar.copy(_V(rdet3, c, [(1, c)]), _V(rdet3, 0, [(1, c)]))
                nc.scalar.copy(_V(rdet3, 2 * c, [(1, c)]), _V(rdet3, 0, [(1, c)]))

                # tneg: t planes 9..11 *= -1 (in place, ACT)
                tp = _V(t, 9 * c, [(1, 3 * c)])
                nc.scalar.mul(tp, tp, -1.0)

                # O row r = Z row r * rdet (in place over Z), flat 3C each
                for r in range(3):
                    eng[plan[f"scale{r}"]].tensor_mul(
                        _V(po, 3 * r * c, [(1, 3 * c)]),
                        _V(po, 3 * r * c, [(1, 3 * c)]),
                        _V(rdet3, 0, [(1, 3 * c)]),
                    )
                # wp row r = O row r * (-t), overwrites Q block
                for r in range(3):
                    eng[plan[f"wp{r}"]].tensor_mul(
                        _V(qq, 3 * r * c, [(1, 3 * c)]),
                        _V(po, 3 * r * c, [(1, 3 * c)]),
                        _V(t, 9 * c, [(1, 3 * c)]),
                    )

                # w_r = wp[3r] + wp[3r+1] + wp[3r+2] -> po planes 9..11
                s = tmp.tile([P, 3 * c], f32, tag="s")
                eng[plan["s"]].tensor_add(
                    _V(s, 0, [(c, 3), (1, c)]),
                    _V(qq, 0, [(3 * c, 3), (1, c)]),
                    _V(qq, c, [(3 * c, 3), (1, c)]),
                )
                eng[plan["w"]].tensor_add(
                    _V(po, 9 * c, [(c, 3), (1, c)]),
                    _V(s, 0, [(c, 3), (1, c)]),
                    _V(qq, 2 * c, [(3 * c, 3), (1, c)]),
                )

                nc.sync.dma_start(out=out_t[n], in_=po[:])

    return nc


_CACHE = {}


def _get_nc():
    if "nc" not in _CACHE:
        nc = build_nc()
        nc.finalize()
        _CACHE["nc"] = nc
    return _CACHE["nc"]


def _shard_inputs(trf):
    """(B,3,4) -> per-core (nch*128, 12*C) planar slabs."""
    x = np.ascontiguousarray(np.asarray(trf, dtype=np.float32)).reshape(
        NCORES, NCH, P, C, 12
    )
    # permute matrix positions into plane slots, planes outer, matrices inner
    xp = x[:, :, :, :, POS].transpose(0, 1, 2, 4, 3)  # (8, nch, 128, 12, C)
    xp = np.ascontiguousarray(xp).reshape(NCORES, NCH * P, 12 * C)
    return xp


def _unshard_output(outs):
    """per-core (nch*128, 12*C) planar -> (B, 3, 4)."""
    o = outs.reshape(NCORES, NCH, P, 12, C).transpose(0, 1, 2, 4, 3)
    full = np.empty((NCORES, NCH, P, C, 12), dtype=np.float32)
    full[..., OPOS] = o
    return full.reshape(B, 3, 4)


def run(trf, trace=False, **spmd_kwargs):
    """Shard, run on 8 cores, gather. Returns (output, BassKernelResults)."""
    from concourse.bass_utils import run_bass_kernel_spmd

    xp = _shard_inputs(trf)
    in_maps = [{"trf": xp[i]} for i in range(NCORES)]
    nc = _get_nc()
    res = run_bass_kernel_spmd(
        nc, in_maps, list(range(NCORES)), trace=trace, **spmd_kwargs
    )
    outs = np.stack([np.asarray(res.results[i]["out"]) for i in range(NCORES)])
    return _unshard_output(outs).astype(np.float32), res


def kernel(trf):
    return run(trf)[0]


# revision 8
# speedup vs baseline: 1.6749x; 1.0021x over previous
"""Trainium2 Bass kernel: batched inverse of homogeneous affine transforms.

Problem: trf (B, 3, 4) fp32 "shift" affines. Padded M = [[I3 + dA, t], [0, 1]].
Output = top 3 rows of M^-1 = [A^-1 | -A^-1 t] where A = I3 + dA.

Closed form via the column-cross-product adjugate:
    Z[3r+j]  = P[3r+j] - Q[3r+j]   (cross(a_{r+1}, a_{r+2}) components)
    det      = a1 . Z[0:3] ; O = Z * (1/det) ; w_r = sum_j O[r][j] * (-t_j)

Layout: PLANAR per partition. Host pre-permutes each core's (BL, 12) slab to
(nch, 128, 12, C): partition p holds 12 contiguous planes of C consecutive
matrices. Every engine op then runs on dense step-1 inner runs (measured ~2x
faster on DVE than the stride-12 interleaved layout) while each DMA still
moves one contiguous 24KB run per partition.

Plane permutation POS (slot -> matrix position) was chosen by combinatorial
search so the 9 Q-products batch as 3 ops, P row 0 batches, and tm batches
(arithmetic-progression plane strides). Work is split DVE / GPSIMD / ACT to
balance engine busy time; all pools are double-buffered so chunks pipeline.
"""

import numpy as np

B = 4_194_304
NCORES = 8
BL = B // NCORES  # 524288 matrices per core
P = 128
C = 512           # matrices per partition per chunk
NCH = BL // (P * C)  # 8 chunks

# slot -> input position (position = 4*r + col, row-major (3,4))
POS = [5, 0, 4, 8, 9, 10, 2, 6, 1, 3, 7, 11]
# output plane k -> output position: planes 0..8 = O[r][j] at 4r+j, 9..11 = w_r
OPOS = [0, 1, 2, 4, 5, 6, 8, 9, 10, 3, 7, 11]

# P products (out plane 3r+j in po block), as (out, in0_slot, in1_slot):
# row 0 batched: out {0,1,2}, in0 [0,4,8] (step 4), in1 [5,6,7] (step 1)
# rows 1,2 as pairs (2-term progressions are always affine):
# (out_base, out_step, in0_base, in0_step, in1_base, in1_step)
P_PAIRS = [
    (3, 1, 7, -2, 3, -2),   # (3,7,3), (4,5,1)
    (5, 1, 6, -4, 2, 2),    # (5,6,2), (6,2,4)
    (7, 1, 3, -2, 8, -8),   # (7,3,8), (8,1,0)
]
# Q products batched by j: (out_base, out_step, in0_base, in0_step, in1_base, in1_step)
Q_BATCH = [
    (0, 3, 7, -2, 4, -2),   # j=0: out {0,3,6}, in0 [7,5,3], in1 [4,2,0]
    (1, 3, 5, -2, 8, -2),   # j=1: out {1,4,7}, in0 [5,3,1], in1 [8,6,4]
    (2, 3, 0, 1, 6, 1),     # j=2: out {2,5,8}, in0 [0,1,2], in1 [6,7,8]
]

# engine plan: op -> "v" (DVE) / "g" (GPSIMD). GPSIMD's SBUF port is shared
# with the DVE (POOL slot): measured combined V+G throughput during overlap
# equals V-alone, so all 2-input work stays on V; ACT (own port) runs 1-input.
DEFAULT_PLAN = {
    "p0": "v",                               # batched P row 0
    **{f"pp{i}": "v" for i in range(3)},     # P pairs (rows 1,2)
    **{f"q{j}": "v" for j in range(3)},      # batched Q
    "z": "v",
    "tm": "v",
    "det1": "v",
    "det2": "v",
    "s": "v",
    "w": "v",
    "scale": "v",
    **{f"wp{r}": "v" for r in range(3)},
}


def _V(base_ap, off, dims):
    """Strided view of a tile: dims = [(step, count), ...] free dims, last
    dim innermost. Offset in elements."""
    import concourse.bass as bass

    return bass.AP(
        base_ap.tensor,
        base_ap.offset + off,
        [list(base_ap.ap[0])] + [[int(s), int(n)] for s, n in dims],
    )


def build_nc(bl=BL, c=C, plan=None):
    import concourse.bass as bass
    import concourse.bacc as bacc
    import concourse.mybir as mybir
    from concourse.tile import TileContext

    plan = dict(DEFAULT_PLAN, **(plan or {}))
    f32 = mybir.dt.float32
    nch = bl // (P * c)
    assert bl == nch * P * c

    nc = bacc.Bacc()
    # DRAM layout (host-permuted): (nch*128, 12*C) — row = (chunk, partition),
    # 12 planar planes of C floats contiguous per row.
    trf = nc.declare_dram_parameter("trf", [nch * P, 12 * c], f32, isOutput=False)
    out = nc.declare_dram_parameter("out", [nch * P, 12 * c], f32, isOutput=True)
    trf_t = trf.ap().rearrange("(n p) f -> n p f", p=P)
    out_t = out.ap().rearrange("(n p) f -> n p f", p=P)

    with TileContext(nc) as tc:
        with (
            tc.tile_pool(name="io", bufs=2) as io,
            tc.tile_pool(name="tmp", bufs=2) as tmp,
        ):
            for n in range(nch):
                eng = {"v": nc.vector, "g": nc.gpsimd}

                t = io.tile([P, 12 * c], f32, tag="t")
                nc.sync.dma_start(out=t[:], in_=trf_t[n])

                # diag += 1: slots {0,1} and {5}
                d01 = _V(t, 0, [(1, 2 * c)])
                nc.scalar.add(d01, d01, 1.0)
                d5 = _V(t, 5 * c, [(1, c)])
                nc.scalar.add(d5, d5, 1.0)

                po = io.tile([P, 12 * c], f32, tag="po")  # P/Z/O planes 0..8, w 9..11
                qq = tmp.tile([P, 9 * c], f32, tag="qq")  # Q then wp

                # P row 0 batched
                eng[plan["p0"]].tensor_mul(
                    _V(po, 0, [(c, 3), (1, c)]),
                    _V(t, 0, [(4 * c, 3), (1, c)]),
                    _V(t, 5 * c, [(c, 3), (1, c)]),
                )
                # P pairs (rows 1, 2)
                for i, (ob, os_, a0, s0, b0, s1) in enumerate(P_PAIRS):
                    eng[plan[f"pp{i}"]].tensor_mul(
                        _V(po, ob * c, [(os_ * c, 2), (1, c)]),
                        _V(t, a0 * c, [(s0 * c, 2), (1, c)]),
                        _V(t, b0 * c, [(s1 * c, 2), (1, c)]),
                    )
                # Q batched by j
                for j, (ob, os_, a0, s0, b0, s1) in enumerate(Q_BATCH):
                    eng[plan[f"q{j}"]].tensor_mul(
                        _V(qq, ob * c, [(os_ * c, 3), (1, c)]),
                        _V(t, a0 * c, [(s0 * c, 3), (1, c)]),
                        _V(t, b0 * c, [(s1 * c, 3), (1, c)]),
                    )

                # Z = P - Q (in place over P block), flat 9C
                pf = _V(po, 0, [(1, 9 * c)])
                eng[plan["z"]].tensor_sub(pf, pf, _V(qq, 0, [(1, 9 * c)]))

                # tm[j] = a1_j * Z_j: in0 slots [1,2,3], in1 Z planes [0,1,2]
                tm = tmp.tile([P, 3 * c], f32, tag="tm")
                eng[plan["tm"]].tensor_mul(
                    _V(tm, 0, [(c, 3), (1, c)]),
                    _V(t, c, [(c, 3), (1, c)]),
                    _V(po, 0, [(c, 3), (1, c)]),
                )
                det = tmp.tile([P, c], f32, tag="det")
                eng[plan["det1"]].tensor_add(
                    det[:], _V(tm, 0, [(1, c)]), _V(tm, c, [(1, c)])
                )
                eng[plan["det2"]].tensor_add(det[:], det[:], _V(tm, 2 * c, [(1, c)]))

                # rdet = 1/det (single custom DVE op, ~4e-6 rel err; det~1 so
                # no edge cases), replicated to 9 planes via log-doubling ACT
                # copies so the scale stage is one flat 9C op
                rdet9 = tmp.tile([P, 9 * c], f32, tag="rdet9")
                nc.vector.reciprocal_approx_fast(_V(rdet9, 0, [(1, c)]), det[:])
                nc.scalar.copy(_V(rdet9, c, [(1, c)]), _V(rdet9, 0, [(1, c)]))
                nc.scalar.copy(_V(rdet9, 2 * c, [(1, 2 * c)]), _V(rdet9, 0, [(1, 2 * c)]))
                nc.scalar.copy(_V(rdet9, 4 * c, [(1, 4 * c)]), _V(rdet9, 0, [(1, 4 * c)]))
                nc.scalar.copy(_V(rdet9, 8 * c, [(1, c)]), _V(rdet9, 0, [(1, c)]))

                # tneg: t planes 9..11 *= -1 (in place, ACT)
                tp = _V(t, 9 * c, [(1, 3 * c)])
                nc.scalar.mul(tp, tp, -1.0)

                # O = Z * rdet (in place over Z), one flat 9C op
                eng[plan["scale"]].tensor_mul(
                    _V(po, 0, [(1, 9 * c)]),
                    _V(po, 0, [(1, 9 * c)]),
                    _V(rdet9, 0, [(1, 9 * c)]),
                )
                # wp row r = O row r * (-t), overwrites Q block
                for r in range(3):
                    eng[plan[f"wp{r}"]].tensor_mul(
                        _V(qq, 3 * r * c, [(1, 3 * c)]),
                        _V(po, 3 * r * c, [(1, 3 * c)]),
                        _V(t, 9 * c, [(1, 3 * c)]),
                    )

                # O block can ship while the w tail computes
                nc.sync.dma_start(
                    out=_V(out_t[n], 0, [(1, 9 * c)]), in_=_V(po, 0, [(1, 9 * c)])
                )

                # w_r = wp[3r] + wp[3r+1] + wp[3r+2] -> po planes 9..11
                # (s scratch reuses tm, dead after the det sums)
                eng[plan["s"]].tensor_add(
                    _V(tm, 0, [(c, 3), (1, c)]),
                    _V(qq, 0, [(3 * c, 3), (1, c)]),
                    _V(qq, c, [(3 * c, 3), (1, c)]),
                )
                eng[plan["w"]].tensor_add(
                    _V(po, 9 * c, [(c, 3), (1, c)]),
                    _V(tm, 0, [(c, 3), (1, c)]),
                    _V(qq, 2 * c, [(3 * c, 3), (1, c)]),
                )

                nc.sync.dma_start(
                    out=_V(out_t[n], 9 * c, [(1, 3 * c)]),
                    in_=_V(po, 9 * c, [(1, 3 * c)]),
                )

    return nc


_CACHE = {}


def _get_nc():
    if "nc" not in _CACHE:
        nc = build_nc()
        nc.finalize()
        _CACHE["nc"] = nc
    return _CACHE["nc"]


def _shard_inputs(trf):
    """(B,3,4) -> per-core (nch*128, 12*C) planar slabs."""
    x = np.ascontiguousarray(np.asarray(trf, dtype=np.float32)).reshape(
        NCORES, NCH, P, C, 12
    )
    # permute matrix positions into plane slots, planes outer, matrices inner
    xp = x[:, :, :, :, POS].transpose(0, 1, 2, 4, 3)  # (8, nch, 128, 12, C)
    xp = np.ascontiguousarray(xp).reshape(NCORES, NCH * P, 12 * C)
    return xp


def _unshard_output(outs):
    """per-core (nch*128, 12*C) planar -> (B, 3, 4)."""
    o = outs.reshape(NCORES, NCH, P, 12, C).transpose(0, 1, 2, 4, 3)
    full = np.empty((NCORES, NCH, P, C, 12), dtype=np.float32)
    full[..., OPOS] = o
    return full.reshape(B, 3, 4)


def run(trf, trace=False, **spmd_kwargs):
    """Shard, run on 8 cores, gather. Returns (output, BassKernelResults)."""
    from concourse.bass_utils import run_bass_kernel_spmd

    xp = _shard_inputs(trf)
    in_maps = [{"trf": xp[i]} for i in range(NCORES)]
    nc = _get_nc()
    res = run_bass_kernel_spmd(
        nc, in_maps, list(range(NCORES)), trace=trace, **spmd_kwargs
    )
    outs = np.stack([np.asarray(res.results[i]["out"]) for i in range(NCORES)])
    return _unshard_output(outs).astype(np.float32), res


def kernel(trf):
    return run(trf)[0]
